# revision 25
# baseline (speedup 1.0000x reference)
"""Trainium2 Bass kernel for a 2-layer Chebyshev GCN (K=3) over a random graph.

Contract: kernel(**inputs) takes the FULL unsharded inputs (as produced by the
problem's setup_inputs) and returns the FULL output [N, out_f] float32.

Strategy (8 NeuronCores, SPMD single NEFF):
  - Nodes are assigned to (core, block, lane) slots by a host-side greedy
    balancer so that every 128-row block receives ~the same number of incident
    edges; all blocks then use a uniform CPB chunks-per-block and the device
    program is a handful of For_i hardware loops (~300 BIR instructions
    instead of ~18k fully unrolled — the per-call walrus compile is the
    dominant wall-clock cost under axon).
  - propagate(T)[r] = -dis[r] * sum_{e: row=r} w_e * (dis*T)[col_e]:
      * the scaled feature table Ts = dis*T lives replicated in DRAM (bf16);
      * per chunk, 128 source rows are fetched with one [128,1]-offset
        indirect DMA gather (offset APs must be physical, so the block's
        offset columns are first staged into a fixed tile);
      * the segment-sum is a one-hot matmul accumulated in PSUM over the
        block's chunks; per-row scale factors are applied afterwards in one
        batched 3D-broadcast vector op over all blocks.
  - Degree/dis vectors are computed on the host (f64) and shipped packed.
  - Cross-core redistribution of new tables is an AllGather; BN statistics
    use a PSUM accumulator over the dense loop plus one AllReduce.
"""

import heapq
import sys

import numpy as np

sys.path.insert(0, "/opt/trn_rl_repo")

import ml_dtypes

BF16 = ml_dtypes.bfloat16


class Meta:
    pass


# ---------------------------------------------------------------------------
# Host-side preprocessing: balance nodes into blocks, pack edges, build inputs
# ---------------------------------------------------------------------------


def _balance_nodes(row, N, n_blocks):
    """Assign each node to one of n_blocks 128-slot blocks, balancing the
    per-block edge (in-degree) totals. Returns (blk_of, lane_of, max_load)."""
    cnt = np.bincount(row, minlength=N).astype(np.int64)
    order = np.argsort(-cnt, kind="stable")
    blk_of = np.empty(N, dtype=np.int64)
    lane_of = np.empty(N, dtype=np.int64)
    load = np.zeros(n_blocks, dtype=np.int64)
    nnode = np.zeros(n_blocks, dtype=np.int64)
    heap = [(0, b) for b in range(n_blocks)]
    heapq.heapify(heap)
    for nd in order:
        while True:
            l, b = heapq.heappop(heap)
            if nnode[b] < 128:
                break
        blk_of[nd] = b
        lane_of[nd] = nnode[b]
        nnode[b] += 1
        load[b] += cnt[nd]
        if nnode[b] < 128:
            heapq.heappush(heap, (load[b], b))
    return blk_of, lane_of, int(load.max())


def _host_prep(x, edge_index, edge_weight, W1, b1, W2, b2, bn_gamma, bn_beta,
               lin_W, lin_b, n_cores=8):
    m = Meta()
    N, in_f = x.shape
    E = edge_index.shape[1]
    m.N, m.E, m.C = int(N), int(E), int(n_cores)
    m.in_f = int(in_f)
    m.c1 = int(W1.shape[2])
    m.c2 = int(W2.shape[2])
    m.out_f = int(lin_W.shape[0])
    m.NB = (N + 128 * n_cores - 1) // (128 * n_cores)   # blocks per core
    m.NP = m.NB * 128                                   # padded rows per core
    m.TN = m.C * m.NP                                   # replicated table rows
    m.F = max(m.in_f, m.c1, m.c2)
    NBG = m.C * m.NB                                    # global block count

    row = np.asarray(edge_index[0], dtype=np.int64)
    col = np.asarray(edge_index[1], dtype=np.int64)
    w = np.asarray(edge_weight, dtype=np.float64)

    blk_of, lane_of, maxload = _balance_nodes(row, m.N, NBG)
    m.CPB = max((maxload + 127) // 128, 1)              # uniform chunks/block
    m.CH = m.NB * m.CPB                                 # chunks per core

    core_of = blk_of // m.NB
    lblk_of = blk_of % m.NB
    slot_of = lblk_of * 128 + lane_of                   # slot within core
    tcol_of = core_of * m.NP + slot_of                  # replicated-table row
    m.core_of, m.slot_of = core_of, slot_of

    # per-slot degree -> dis vectors (host, f64)
    deg = np.bincount(row, weights=w, minlength=m.N)
    with np.errstate(divide="ignore"):
        rs = np.where(deg > 0, 1.0 / np.sqrt(np.maximum(deg, 1e-300)), 0.0)
    rinv = np.where(deg > 0, 1.0 / np.maximum(deg, 1e-300), 0.0)

    # edge placement: sort by destination global block, sequential fill
    gblk = blk_of[row]
    order = np.argsort(gblk, kind="stable")
    gblk_s = gblk[order]
    starts = np.searchsorted(gblk_s, np.arange(NBG + 1))
    pos = np.arange(E, dtype=np.int64) - starts[gblk_s]
    assert pos.max() < m.CPB * 128
    chunk = pos // 128
    lane = pos % 128
    ecore = gblk_s // m.NB
    col_flat = np.zeros((m.C, 128, m.CH), dtype=np.int32)
    w_flat = np.zeros((m.C, 128, m.CH), dtype=np.float32)
    d_flat = np.zeros((m.C, 128, m.CH), dtype=np.float32)
    ccol = (gblk_s % m.NB) * m.CPB + chunk
    col_flat[ecore, lane, ccol] = tcol_of[col[order]]
    w_flat[ecore, lane, ccol] = w[order]
    d_flat[ecore, lane, ccol] = lane_of[row[order]]

    # packed f32 consts: b1rep / b2rep / linbrep [128, c], then gamma /
    # beta rows (row 0 only).
    NB = m.NB
    m.O_B1 = 0
    m.O_B2 = m.O_B1 + m.c1
    m.O_LINB = m.O_B2 + m.c2
    m.O_GAM = m.O_LINB + m.out_f
    m.O_BET = m.O_GAM + m.c1
    m.W_CF32 = m.O_BET + m.c1

    cf32 = np.zeros((m.C, 1, m.W_CF32), dtype=np.float32)
    cf32[:, 0, m.O_B1:m.O_B1 + m.c1] = np.asarray(b1, np.float32)
    cf32[:, 0, m.O_B2:m.O_B2 + m.c2] = np.asarray(b2, np.float32)
    cf32[:, 0, m.O_LINB:m.O_LINB + m.out_f] = np.asarray(lin_b, np.float32)
    cf32[:, 0, m.O_GAM:m.O_GAM + m.c1] = np.asarray(bn_gamma, np.float32)
    cf32[:, 0, m.O_BET:m.O_BET + m.c1] = np.asarray(bn_beta, np.float32)

    # packed bf16 consts: dis / negdis / negdis2 / negdisx2 / vmask
    # [128, NB] each (slot-major: v[p, b] for slot b*128+p; converted to
    # f32 on device). id128 / iota / ones are generated on device.
    m.O_DIS = 0
    m.O_NEG = m.O_DIS + NB
    m.O_NEG2 = m.O_NEG + NB
    m.O_NEGX2 = m.O_NEG2 + NB
    m.O_VM = m.O_NEGX2 + NB
    m.W_CBF = m.O_VM + NB

    def slotv(vals_per_node, fill=0.0):
        a = np.full((m.C, m.NP), fill, dtype=np.float64)
        a[core_of, slot_of] = vals_per_node
        return a.reshape(m.C, m.NB, 128).transpose(0, 2, 1)  # [C, 128, NB]

    cbf = np.zeros((m.C, 128, m.W_CBF), dtype=np.float32)
    cbf[:, :, m.O_DIS:m.O_DIS + NB] = slotv(rs)
    cbf[:, :, m.O_NEG:m.O_NEG + NB] = slotv(-rs)
    cbf[:, :, m.O_NEG2:m.O_NEG2 + NB] = slotv(-rinv)
    cbf[:, :, m.O_NEGX2:m.O_NEGX2 + NB] = slotv(-2.0 * rs)
    cbf[:, :, m.O_VM:m.O_VM + NB] = slotv(1.0)
    cbf = cbf.astype(BF16)

    # packed bf16 weights: W1 (3 x [in_f, c1]) | W2 (3 x [c1, c2]) | lin_W.T
    m.O_W1, m.O_W2 = 0, 3 * m.c1
    m.O_LW = m.O_W2 + 3 * m.c2
    m.W_WP = m.O_LW + m.out_f
    m.P_WP = max(m.in_f, m.c1, m.c2)
    wp = np.zeros((m.P_WP, m.W_WP), dtype=np.float32)
    for k in range(3):
        wp[:m.in_f, m.O_W1 + k * m.c1:m.O_W1 + (k + 1) * m.c1] = \
            np.asarray(W1, np.float32)[k]
        wp[:m.c1, m.O_W2 + k * m.c2:m.O_W2 + (k + 1) * m.c2] = \
            np.asarray(W2, np.float32)[k]
    wp[:m.c2, m.O_LW:m.O_LW + m.out_f] = np.asarray(lin_W, np.float32).T
    wp = wp.astype(BF16)

    xf = np.asarray(x, np.float32)
    in_maps = []
    for c in range(m.C):
        xp = np.zeros((m.NP, m.in_f), dtype=np.float32)
        mask_c = core_of == c
        xp[slot_of[mask_c]] = xf[mask_c]
        in_maps.append({
            "xs": xp.astype(BF16),
            "collo": np.ascontiguousarray(
                (col_flat[c] & 0xFFFF).astype(np.uint16)),
            "colhi": np.ascontiguousarray(
                (col_flat[c] >> 16).astype(np.uint8)),
            "wsb": np.ascontiguousarray(w_flat[c]).astype(BF16),
            "dsb": np.ascontiguousarray(d_flat[c]).astype(np.uint8),
            "cf32": np.ascontiguousarray(cf32[c]),
            "cbf": np.ascontiguousarray(cbf[c]),
            "wp": wp,
        })
    return m, in_maps


def _assemble(m, results):
    """Gather per-core bf16 outputs back to the full [N, out_f] f32 array."""
    allout = np.concatenate(
        [np.asarray(r["out"], dtype=np.float32) for r in results], axis=0)
    out = np.empty((m.N, m.out_f), dtype=np.float32)
    out[np.arange(m.N)] = allout[m.core_of * m.NP + m.slot_of]
    return out


# ---------------------------------------------------------------------------
# Device program
# ---------------------------------------------------------------------------


def _build_program(m):
    import concourse.bass as bass
    import concourse.tile as tile
    from concourse import bacc, mybir

    f32 = mybir.dt.float32
    bf16 = mybir.dt.bfloat16
    i32 = mybir.dt.int32

    nc = bacc.Bacc(num_devices=m.C, num_swdge_queues=4)

    u8 = mybir.dt.uint8
    u16 = mybir.dt.uint16
    T = {}
    T["xs"] = nc.dram_tensor("xs", [m.NP, m.in_f], bf16, kind="ExternalInput")
    T["collo"] = nc.dram_tensor("collo", [128, m.CH], u16,
                                kind="ExternalInput")
    T["colhi"] = nc.dram_tensor("colhi", [128, m.CH], u8,
                                kind="ExternalInput")
    T["wsb"] = nc.dram_tensor("wsb", [128, m.CH], bf16, kind="ExternalInput")
    T["dsb"] = nc.dram_tensor("dsb", [128, m.CH], u8, kind="ExternalInput")
    T["cf32"] = nc.dram_tensor("cf32", [1, m.W_CF32], f32,
                               kind="ExternalInput")
    T["cbf"] = nc.dram_tensor("cbf", [128, m.W_CBF], bf16,
                              kind="ExternalInput")
    T["wp"] = nc.dram_tensor("wp", [m.P_WP, m.W_WP], bf16,
                             kind="ExternalInput")
    T["out"] = nc.dram_tensor("out", [m.NP, m.out_f], bf16,
                              kind="ExternalOutput")

    with tile.TileContext(nc) as tc:
        _emit(nc, tc, m, T)
    nc.finalize()
    return nc


def _emit(nc, tc, m, T):
    from contextlib import ExitStack

    import concourse.bass as bass
    from concourse import mybir
    from concourse.bass import ds

    f32 = mybir.dt.float32
    bf16 = mybir.dt.bfloat16
    i32 = mybir.dt.int32
    OP = mybir.AluOpType
    rg = [list(range(m.C))]
    NB, CPB, F = m.NB, m.CPB, m.F
    c1, c2, in_f, out_f = m.c1, m.c2, m.in_f, m.out_f

    with ExitStack() as ctx:
        cp = ctx.enter_context(tc.tile_pool(name="consts", bufs=1))
        bigp = ctx.enter_context(tc.tile_pool(name="big", bufs=1))
        gp = ctx.enter_context(tc.tile_pool(name="gth", bufs=4))
        ep = ctx.enter_context(tc.tile_pool(name="epi", bufs=4))
        pp = ctx.enter_context(tc.tile_pool(name="ps", bufs=2, space="PSUM"))
        psp = ctx.enter_context(tc.tile_pool(name="pstat", bufs=1,
                                             space="PSUM"))
        dp = ctx.enter_context(tc.tile_pool(name="dram", bufs=1, space="DRAM"))

        def load_const(name, shape, dtype):
            t = cp.tile(shape, dtype, tag=name, name=name)
            nc.sync.dma_start(out=t[:], in_=T[name][:])
            return t

        u8 = mybir.dt.uint8
        u16 = mybir.dt.uint16
        collo_s = load_const("collo", [128, m.CH], u16)
        colhi_s = load_const("colhi", [128, m.CH], u8)
        d8_s = load_const("dsb", [128, m.CH], u8)
        w_s = load_const("wsb", [128, m.CH], bf16)
        cf = load_const("cf32", [1, m.W_CF32], f32)
        cb = load_const("cbf", [128, m.W_CBF], bf16)
        wp = load_const("wp", [m.P_WP, m.W_WP], bf16)

        # unpack edge indices (u16 lo + u8 hi -> i32) and lane codes (u8->bf16)
        col_s = cp.tile([128, m.CH], i32, tag="col_s", name="col_s")
        d_s = cp.tile([128, m.CH], bf16, tag="d_s", name="d_s")
        with tc.tile_pool(name="unpack", bufs=1) as up:
            lof = up.tile([128, m.CH], f32, tag="lof", name="lof")
            hif = up.tile([128, m.CH], f32, tag="hif", name="hif")
            nc.vector.tensor_copy(out=lof[:], in_=collo_s[:])
            nc.vector.tensor_copy(out=hif[:], in_=colhi_s[:])
            nc.vector.tensor_scalar(out=hif[:], in0=hif[:], scalar1=65536.0,
                                    scalar2=None, op0=OP.mult)
            nc.vector.tensor_tensor(out=hif[:], in0=hif[:], in1=lof[:],
                                    op=OP.add)
            nc.vector.tensor_copy(out=col_s[:], in_=hif[:])
            nc.vector.tensor_copy(out=d_s[:], in_=d8_s[:])

        # dis / negdis / negdis2 / negdisx2 / vmask -> f32 scratch
        dv = cp.tile([128, 5 * NB], f32, tag="dv", name="dv")
        nc.vector.tensor_copy(out=dv[:], in_=cb[:, m.O_DIS:m.O_DIS + 5 * NB])

        # device-generated constants: identity, iota row, ones col/row
        from concourse.masks import make_identity
        id_t = cp.tile([128, 128], bf16, tag="id_t", name="id_t")
        make_identity(nc, id_t[:])
        iota_t = cp.tile([128, 128], bf16, tag="iota_t", name="iota_t")
        nc.gpsimd.iota(out=iota_t[:], pattern=[[1, 128]], base=0,
                       channel_multiplier=0,
                       allow_small_or_imprecise_dtypes=True)
        oner_t = cp.tile([1, 128], bf16, tag="oner_t", name="oner_t")
        nc.vector.memset(oner_t[:], 1.0)
        dis = dv[:, 0:NB]
        negdis = dv[:, NB:2 * NB]
        negdis2 = dv[:, 2 * NB:3 * NB]
        negdisx2 = dv[:, 3 * NB:4 * NB]
        vmf = dv[:, 4 * NB:5 * NB]
        gam = cf[0:1, m.O_GAM:m.O_GAM + c1]
        bet = cf[0:1, m.O_BET:m.O_BET + c1]
        id_s = id_t[:]
        iota_s = iota_t[:]
        oner = oner_t[:]

        # replicate bias rows across partitions (ones-row outer product)
        cfb = cp.tile([1, m.W_CF32], bf16, tag="cfb", name="cfb")
        nc.vector.tensor_copy(out=cfb[:], in_=cf[:])
        brep_t = cp.tile([128, c1 + c2 + out_f], f32, tag="brep_t",
                         name="brep_t")
        for off, wdt, o2 in ((m.O_B1, c1, 0), (m.O_B2, c2, c1),
                             (m.O_LINB, out_f, c1 + c2)):
            rp = pp.tile([128, F], f32, tag="dense", name="dense")
            nc.tensor.matmul(out=rp[:, :wdt], lhsT=oner,
                             rhs=cfb[:, off:off + wdt], start=True, stop=True)
            nc.scalar.copy(out=brep_t[:, o2:o2 + wdt], in_=rp[:, :wdt])
        b1r = brep_t[:, 0:c1]
        b2r = brep_t[:, c1:c1 + c2]
        linbr = brep_t[:, c1 + c2:c1 + c2 + out_f]
        w1 = [wp[:in_f, m.O_W1 + k * c1:m.O_W1 + (k + 1) * c1]
              for k in range(3)]
        w2 = [wp[:c1, m.O_W2 + k * c2:m.O_W2 + (k + 1) * c2]
              for k in range(3)]
        linwt = wp[:c2, m.O_LW:m.O_LW + out_f]

        # f32 ones column for the (f32) stats matmuls
        onescol = cp.tile([128, 1], f32, tag="onescol", name="onescol")
        nc.vector.memset(onescol[:], 1.0)

        def bigtile(tag, f, dtype):
            return bigp.tile([128, NB * f], dtype, tag=tag, name=tag)

        x_sb = bigtile("x", in_f, f32)
        nc.gpsimd.dma_start(
            out=x_sb[:].rearrange("p (b f) -> p b f", b=NB),
            in_=T["xs"][:].rearrange("(b p) f -> p b f", p=128))

        stage = bigtile("stage", F, bf16)

        sh = [dp.tile([m.NP, in_f], bf16, tag="sh0", name="sh0"),
              dp.tile([m.NP, in_f], bf16, tag="sh1", name="sh1"),
              dp.tile([m.NP, c1], bf16, tag="sh2", name="sh2"),
              dp.tile([m.NP, c1], bf16, tag="sh3", name="sh3")]
        tb = [dp.tile([m.TN, in_f], bf16, tag="tb0", name="tb0",
                      addr_space="Shared"),
              dp.tile([m.TN, in_f], bf16, tag="tb1", name="tb1",
                      addr_space="Shared"),
              dp.tile([m.TN, c1], bf16, tag="tb2", name="tb2",
                      addr_space="Shared"),
              dp.tile([m.TN, c1], bf16, tag="tb3", name="tb3",
                      addr_space="Shared")]

        def stage_to_table(i, f):
            nc.sync.dma_start(
                out=sh[i][:].rearrange("(b p) f -> p b f", p=128),
                in_=stage[:, :NB * f].rearrange("p (b f) -> p b f", b=NB))
            nc.gpsimd.collective_compute(
                "AllGather", OP.bypass, replica_groups=rg,
                ins=[sh[i][:]], outs=[tb[i][:]])

        def bmul(out_ap, in_ap, vec, f):
            """out[:, b*f:(b+1)*f] = in[:, b*f:(b+1)*f] * vec[:, b] batched."""
            nc.vector.tensor_tensor(
                out=out_ap.rearrange("p (b f) -> p b f", b=NB),
                in0=in_ap.rearrange("p (b f) -> p b f", b=NB),
                in1=vec.unsqueeze(2).broadcast_to([128, NB, f]),
                op=OP.mult)

        # table0 = dis * x
        bmul(stage[:, :NB * in_f], x_sb[:], dis, in_f)
        stage_to_table(0, in_f)

        qctr = [0]

        def propagate(table, f, raw):
            """raw[:, b*f:(b+1)*f] = per-block scatter sums (f32)."""
            with tc.For_i(0, NB, 1) as b:
                colstg = gp.tile([128, CPB], i32, tag="colstg", name="colstg")
                nc.vector.tensor_copy(out=colstg[:],
                                      in_=col_s[:, ds(b * CPB, CPB)])
                g = gp.tile([128, CPB * F], bf16, tag="g", name="g")
                for j in range(CPB):
                    inst = nc.gpsimd.indirect_dma_start(
                        out=g[:, j * f:j * f + f], out_offset=None,
                        in_=table[:],
                        in_offset=bass.IndirectOffsetOnAxis(
                            ap=colstg[:, j:j + 1], axis=0))
                    qn = qctr[0] % 4
                    qctr[0] += 1
                    if qn:
                        inst.ins.queue = f"qPoolDynamic{qn}"
                gw = gp.tile([128, CPB * F], bf16, tag="gw", name="gw")
                nc.vector.tensor_tensor(
                    out=gw[:, :CPB * f].rearrange("p (c f) -> p c f", c=CPB),
                    in0=g[:, :CPB * f].rearrange("p (c f) -> p c f", c=CPB),
                    in1=w_s[:, ds(b * CPB, CPB)].unsqueeze(2)
                        .broadcast_to([128, CPB, f]),
                    op=OP.mult)
                o = gp.tile([128, CPB * 128], bf16, tag="o", name="o")
                nc.vector.tensor_tensor(
                    out=o[:].rearrange("p (c k) -> p c k", c=CPB),
                    in0=iota_s.unsqueeze(1).broadcast_to([128, CPB, 128]),
                    in1=d_s[:, ds(b * CPB, CPB)].unsqueeze(2)
                        .broadcast_to([128, CPB, 128]),
                    op=OP.is_equal)
                psum = pp.tile([128, F], f32, tag="prop", name="prop")
                for j in range(CPB):
                    nc.tensor.matmul(
                        out=psum[:, :f],
                        lhsT=o[:, j * 128:(j + 1) * 128],
                        rhs=gw[:, j * f:j * f + f],
                        start=(j == 0), stop=(j == CPB - 1))
                nc.scalar.copy(out=raw[:, ds(b * f, f)], in_=psum[:, :f])

        # ---------------- conv1 ----------------
        raw = bigtile("raw", F, f32)
        T1 = bigtile("T1", in_f, bf16)
        propagate(tb[0][:], in_f, raw)
        bmul(T1[:], raw[:, :NB * in_f], negdis, in_f)
        bmul(stage[:, :NB * in_f], raw[:, :NB * in_f], negdis2, in_f)
        stage_to_table(1, in_f)

        T2 = bigtile("T2", in_f, bf16)
        propagate(tb[1][:], in_f, raw)
        bmul(raw[:, :NB * in_f], raw[:, :NB * in_f], negdisx2, in_f)
        nc.vector.tensor_tensor(out=T2[:], in0=raw[:, :NB * in_f],
                                in1=x_sb[:], op=OP.subtract)

        # dense conv1: h = vmask*relu(T0@W0+T1@W1+T2@W2+b1), BN stats in PSUM
        h_sb = bigtile("h", c1, f32)
        s1 = psp.tile([1, c1], f32, tag="s1", name="s1")
        s2 = psp.tile([1, c1], f32, tag="s2", name="s2")
        nc.vector.memset(s1[:], 0.0)
        nc.vector.memset(s2[:], 0.0)

        def dense3(srcs, src_f, ws, fin, fout, b):
            hp = pp.tile([128, F], f32, tag="dense", name="dense")
            for k in range(3):
                cbt = ep.tile([128, F], bf16, tag="cast", name="cast")
                nc.scalar.copy(out=cbt[:, :fin],
                               in_=srcs[k][:, ds(b * fin, fin)])
                tp = pp.tile([F, 128], bf16, tag="tp", name="tp")
                nc.tensor.transpose(out=tp[:fin, :], in_=cbt[:, :fin],
                                    identity=id_s)
                tT = ep.tile([F, 128], bf16, tag="tT", name="tT")
                nc.scalar.copy(out=tT[:fin, :], in_=tp[:fin, :])
                nc.tensor.matmul(out=hp[:, :fout], lhsT=tT[:fin, :],
                                 rhs=ws[k], start=(k == 0), stop=(k == 2))
            return hp

        with tc.For_i(0, NB, 1) as b:
            hp = dense3([x_sb, T1, T2], in_f, w1, in_f, c1, b)
            hblk = ep.tile([128, c1], f32, tag="hblk", name="hblk")
            # custom-DVE op (vs plain add): with ant_custom_dve_ops non-empty
            # the compile reuses the in-process dve-table cache instead of
            # regenerating the default table (~0.45s) on every call.
            nc.vector.affine_then_add(out=hblk[:], in0=hp[:, :c1], in1=b1r,
                                      scale=1.0, bias=0.0)
            nc.vector.tensor_scalar(out=hblk[:], in0=hblk[:], scalar1=0.0,
                                    scalar2=None, op0=OP.max)
            vstg = ep.tile([128, 1], f32, tag="vstg", name="vstg")
            nc.vector.tensor_copy(out=vstg[:], in_=dv[:, ds(4 * NB + b, 1)])
            nc.scalar.mul(out=hblk[:], in_=hblk[:], mul=vstg[:])
            nc.scalar.copy(out=h_sb[:, ds(b * c1, c1)], in_=hblk[:])
            hsq = ep.tile([128, c1], f32, tag="sq", name="sq")
            nc.scalar.square(out=hsq[:], in_=hblk[:])
            nc.tensor.matmul(out=s1[:], lhsT=onescol[:], rhs=hblk[:],
                             start=False, stop=False)
            nc.tensor.matmul(out=s2[:], lhsT=onescol[:], rhs=hsq[:],
                             start=False, stop=False)

        # ---------------- BatchNorm ----------------
        stats_sb = cp.tile([1, 2 * c1], f32, tag="stats_sb", name="stats_sb")
        nc.vector.tensor_copy(out=stats_sb[:, :c1], in_=s1[:])
        nc.vector.tensor_copy(out=stats_sb[:, c1:], in_=s2[:])
        st_l = dp.tile([1, 2 * c1], f32, tag="st_l", name="st_l")
        st_g = dp.tile([1, 2 * c1], f32, tag="st_g", name="st_g",
                       addr_space="Shared")
        nc.sync.dma_start(out=st_l[:], in_=stats_sb[:])
        nc.gpsimd.collective_compute("AllReduce", OP.add, replica_groups=rg,
                                     ins=[st_l[:]], outs=[st_g[:]])
        gst = cp.tile([1, 2 * c1], f32, tag="gst", name="gst")
        nc.sync.dma_start(out=gst[:], in_=st_g[:])

        def row(tag):
            return cp.tile([1, c1], f32, tag=tag, name=tag)

        mu, ex2, var, vrec, vrs, gprow, bprow = (row(t) for t in
            ("mu", "ex2", "var", "vrec", "vrs", "gprow", "bprow"))
        inv_n = 1.0 / float(m.N)
        nc.vector.tensor_scalar(out=mu[:], in0=gst[:, :c1], scalar1=inv_n,
                                scalar2=None, op0=OP.mult)
        nc.vector.tensor_scalar(out=ex2[:], in0=gst[:, c1:], scalar1=inv_n,
                                scalar2=None, op0=OP.mult)
        nc.vector.tensor_tensor(out=var[:], in0=mu[:], in1=mu[:], op=OP.mult)
        nc.vector.tensor_tensor(out=var[:], in0=ex2[:], in1=var[:],
                                op=OP.subtract)
        nc.vector.tensor_scalar(out=var[:], in0=var[:], scalar1=1e-5,
                                scalar2=None, op0=OP.add)
        nc.vector.reciprocal(out=vrec[:], in_=var[:])
        nc.scalar.sqrt(out=vrs[:], in_=vrec[:])
        nc.vector.tensor_tensor(out=gprow[:], in0=gam, in1=vrs[:], op=OP.mult)
        nc.vector.tensor_tensor(out=bprow[:], in0=mu[:], in1=gprow[:],
                                op=OP.mult)
        nc.vector.tensor_tensor(out=bprow[:], in0=bet, in1=bprow[:],
                                op=OP.subtract)
        gprow_bf = cp.tile([1, c1], bf16, tag="gprow_bf", name="gprow_bf")
        bprow_bf = cp.tile([1, c1], bf16, tag="bprow_bf", name="bprow_bf")
        nc.vector.tensor_copy(out=gprow_bf[:], in_=gprow[:])
        nc.vector.tensor_copy(out=bprow_bf[:], in_=bprow[:])
        grep = cp.tile([128, c1], f32, tag="grep", name="grep")
        brep = cp.tile([128, c1], f32, tag="brep", name="brep")
        for rowv, rep in ((gprow_bf, grep), (bprow_bf, brep)):
            rp = pp.tile([128, F], f32, tag="dense", name="dense")
            nc.tensor.matmul(out=rp[:, :c1], lhsT=oner, rhs=rowv[:],
                             start=True, stop=True)
            nc.scalar.copy(out=rep[:], in_=rp[:, :c1])

        # h' = g'*h + b' (batched, in place), table2 = dis*h'
        nc.vector.tensor_tensor(
            out=h_sb[:].rearrange("p (b f) -> p b f", b=NB),
            in0=h_sb[:].rearrange("p (b f) -> p b f", b=NB),
            in1=grep[:].unsqueeze(1).broadcast_to([128, NB, c1]), op=OP.mult)
        nc.vector.tensor_tensor(
            out=h_sb[:].rearrange("p (b f) -> p b f", b=NB),
            in0=h_sb[:].rearrange("p (b f) -> p b f", b=NB),
            in1=brep[:].unsqueeze(1).broadcast_to([128, NB, c1]), op=OP.add)
        bmul(stage[:, :NB * c1], h_sb[:], dis, c1)
        stage_to_table(2, c1)

        # ---------------- conv2 ----------------
        T1p = bigtile("T1p", c1, bf16)
        propagate(tb[2][:], c1, raw)
        bmul(T1p[:], raw[:, :NB * c1], negdis, c1)
        bmul(stage[:, :NB * c1], raw[:, :NB * c1], negdis2, c1)
        stage_to_table(3, c1)

        T2p = bigtile("T2p", c1, bf16)
        propagate(tb[3][:], c1, raw)
        bmul(raw[:, :NB * c1], raw[:, :NB * c1], negdisx2, c1)
        nc.vector.tensor_tensor(out=T2p[:], in0=raw[:, :NB * c1],
                                in1=h_sb[:], op=OP.subtract)

        # dense conv2 + final linear
        out_sb = bigtile("out_sb", out_f, bf16)
        with tc.For_i(0, NB, 1) as b:
            hp = dense3([h_sb, T1p, T2p], c1, w2, c1, c2, b)
            h2 = ep.tile([128, c2], f32, tag="h2", name="h2")
            nc.vector.tensor_tensor(out=h2[:], in0=hp[:, :c2], in1=b2r,
                                    op=OP.add)
            nc.vector.tensor_scalar(out=h2[:], in0=h2[:], scalar1=0.0,
                                    scalar2=None, op0=OP.max)
            h2b = ep.tile([128, c2], bf16, tag="h2b", name="h2b")
            nc.scalar.copy(out=h2b[:], in_=h2[:])
            tp = pp.tile([F, 128], bf16, tag="tp", name="tp")
            nc.tensor.transpose(out=tp[:c2, :], in_=h2b[:], identity=id_s)
            h2T = ep.tile([F, 128], bf16, tag="tT", name="tT")
            nc.scalar.copy(out=h2T[:c2, :], in_=tp[:c2, :])
            op_ps = pp.tile([128, out_f], f32, tag="prop", name="prop")
            nc.tensor.matmul(out=op_ps[:], lhsT=h2T[:c2, :], rhs=linwt,
                             start=True, stop=True)
            ob = ep.tile([128, out_f], bf16, tag="ob", name="ob")
            nc.vector.tensor_tensor(out=ob[:], in0=op_ps[:], in1=linbr,
                                    op=OP.add)
            nc.scalar.copy(out=out_sb[:, ds(b * out_f, out_f)], in_=ob[:])
        nc.sync.dma_start(
            out=T["out"][:].rearrange("(b p) f -> p b f", p=128),
            in_=out_sb[:].rearrange("p (b f) -> p b f", b=NB))


# ---------------------------------------------------------------------------
# Entry point
# ---------------------------------------------------------------------------


def _run(inputs, n_cores=8, trace=False):
    from concourse.bass_utils import run_bass_kernel_spmd

    m, in_maps = _host_prep(n_cores=n_cores, **inputs)
    nc = _build_program(m)
    res = run_bass_kernel_spmd(nc, in_maps, core_ids=list(range(n_cores)),
                               trace=trace)
    return _assemble(m, res.results), res


def kernel(**inputs):
    out, _ = _run(inputs, n_cores=8, trace=False)
    return out
